# revision 3
# baseline (speedup 1.0000x reference)
"""Trainium2 Bass kernel for nn_JointGenerator — tensor-parallel v3.

Tensor-parallel over the hidden dim (64 h-dims per core), full batch 128
as the moving operand.  Per step the six cells are retimed so both
dependency cycles contain only TWO AllGather legs:

  step t:  A:  d1(t-1), c0(t)      -> AG1 {d1(t-1), c0(t)}
           fc_d(t-2)
           B1: c2(t-1)             -> AGc2 {c2(t-1)}
           C:  c1(t), d0(t)        -> AG2 {c1(t), d0(t)}   [after AG1 ret]
           B2: d2(t-1)             -> AGd2 {d2(t-1)}       [after AGc2 ret]
           fc_c(t-1)

Main cycle: AG1 -> C -> AG2 -> A(t+1) -> AG1  (2 legs)
Sub cycle:  AGc2 -> B2 -> AGd2 -> B1(t+1) -> AGc2  (2 legs, parallel)

Gather returns are single full-128-partition DMAs per cell (the
baseline's 64-partition half-transfers were the main overhead).
Elementwise uses the baseline's proven pair-reduce (rsb matmul) trick.
gamma == 0 => attention is identity; host numpy fallback otherwise.
"""

import numpy as np
import ml_dtypes

import concourse.bass as bass
import concourse.bacc as bacc
import concourse.mybir as mybir
import concourse.tile as tile
from concourse.bass_utils import run_bass_kernel_spmd

B = 128
T_FULL = 256
H = 512
NCORES = 8
CH = H // NCORES  # 64 h-dims per core

CELLS = ["c0", "c1", "c2", "d0", "d1", "d2"]
NK = {"c0": 9, "c1": 12, "c2": 12, "d0": 9, "d1": 12, "d2": 12}

bf16 = mybir.dt.bfloat16
f32 = mybir.dt.float32
AF = mybir.ActivationFunctionType


def build_kernel(T=T_FULL):
    nc = bacc.Bacc("TRN2", target_bir_lowering=False, debug=False,
                   num_devices=NCORES)

    xc = nc.dram_tensor("xc", [T, 128, B], bf16, kind="ExternalInput")
    xd = nc.dram_tensor("xd", [T, 128, B], bf16, kind="ExternalInput")
    wdr = {c: nc.dram_tensor(f"w_{c}", [NK[c], 128, 2, 128], bf16,
                             kind="ExternalInput") for c in CELLS}
    fcw = {s: nc.dram_tensor(f"fcw_{s}", [4, 128, 2, 128], bf16,
                             kind="ExternalInput") for s in "cd"}
    fcb = {s: nc.dram_tensor(f"fcb_{s}", [128, 2], f32,
                             kind="ExternalInput") for s in "cd"}
    ridm = nc.dram_tensor("ridm", [128, 64], f32, kind="ExternalInput")
    zout = {s: nc.dram_tensor(f"z_{s}", [T, 256, B], f32,
                              kind="ExternalOutput") for s in "cd"}

    # persistent SBUF
    wsb = {c: nc.alloc_sbuf_tensor(f"wsb_{c}", [128, NK[c] * 2 * 128], bf16)
           for c in CELLS}
    fcwsb = {s: nc.alloc_sbuf_tensor(f"fcwsb_{s}", [128, 4 * 2 * 128], bf16)
             for s in "cd"}
    fcbsb = {s: nc.alloc_sbuf_tensor(f"fcbsb_{s}", [128, 2], f32)
             for s in "cd"}
    hsb = {c: nc.alloc_sbuf_tensor(f"h_{c}", [128, 512], bf16) for c in CELLS}
    # per-cell Q tile: [0:64] = c-state (f32, persistent), [64:128] = tanh(c~)
    qsb = {c: nc.alloc_sbuf_tensor(f"q_{c}", [128, 128], f32) for c in CELLS}
    rsb = nc.alloc_sbuf_tensor("rsb", [128, 64], f32)

    with tile.TileContext(nc) as tc:
        with (
            tc.tile_pool(name="xp", bufs=3) as xp,
            tc.tile_pool(name="ps", bufs=5, space="PSUM") as psp,
            tc.tile_pool(name="cnp", bufs=2, space="PSUM") as cnpp,
            tc.tile_pool(name="ew", bufs=3) as ewp,
            tc.tile_pool(name="osb", bufs=3) as osbp,
            tc.tile_pool(name="dr", bufs=3, space="DRAM") as drp,
        ):
            # prologue: weights + state init
            for c in CELLS:
                nc.sync.dma_start(
                    wsb[c][:, :].rearrange("p (k m j) -> p k m j",
                                           k=NK[c], m=2, j=128),
                    wdr[c].ap().rearrange("k p m j -> p k m j"))
                nc.vector.memset(hsb[c][:, :], 0.0)
            for s in "cd":
                nc.sync.dma_start(
                    fcwsb[s][:, :].rearrange("p (k m j) -> p k m j",
                                             k=4, m=2, j=128),
                    fcw[s].ap().rearrange("k p m j -> p k m j"))
                nc.sync.dma_start(fcbsb[s][:, :], fcb[s].ap())
            for c in CELLS:
                nc.vector.memset(qsb[c][:, :], 0.0)
            nc.sync.dma_start(rsb[:, :], ridm.ap())

            def h_tiles(cell):
                return [hsb[cell][:, 128 * j:128 * (j + 1)] for j in range(4)]

            def cell_mms(psum, cell, rhs_tiles, order=None):
                # kt-outer so ready (non-AG-gated) contraction tiles issue
                # first; `order` permutes kt to put ready tiles up front.
                nk = NK[cell]
                assert len(rhs_tiles) == nk
                order = list(range(nk)) if order is None else order
                for m in (0, 1):
                    for pos, kt in enumerate(order):
                        col = (kt * 2 + m) * 128
                        nc.tensor.matmul(
                            psum[:, 128 * m:128 * (m + 1)],
                            wsb[cell][:, col:col + 128],
                            rhs_tiles[kt],
                            start=(pos == 0), stop=(pos == nk - 1))

            def cell_ew(psum, cell, tag):
                # psum: [f;i] in cols 0:128, [o;c~] in cols 128:256
                S = ewp.tile([128, 128], f32, name=f"S{tag}", tag=f"S{tag}")
                O = ewp.tile([64, 128], f32, name=f"O{tag}", tag=f"O{tag}")
                tcn = ewp.tile([64, 128], f32, name=f"tc{tag}",
                               tag=f"tc{tag}")
                P = ewp.tile([128, 128], f32, name=f"P{tag}", tag=f"P{tag}")
                ag = ewp.tile([64, 128], bf16, name=f"ag{tag}",
                              tag=f"ag{tag}")
                cnp = cnpp.tile([64, 128], f32, name=f"cn{tag}", tag="cn")
                nc.scalar.activation(S[:, :], psum[:, 0:128], AF.Sigmoid)
                nc.scalar.activation(O[:, :], psum[0:64, 128:256], AF.Sigmoid)
                nc.scalar.activation(qsb[cell][64:128, :],
                                     psum[64:128, 128:256], AF.Tanh)
                nc.vector.tensor_mul(P[:, :], S[:, :], qsb[cell][:, :])
                # c_next = sig(f)*c + sig(i)*tanh(c~): partition-pair reduce
                nc.tensor.matmul(cnp[:, :], rsb[:, :], P[:, :],
                                 start=True, stop=True)
                nc.vector.tensor_copy(qsb[cell][0:64, :], cnp[:, :])
                nc.scalar.activation(tcn[:, :], cnp[:, :], AF.Tanh)
                nc.vector.tensor_mul(ag[:, :], O[:, :], tcn[:, :])
                return ag

            def exchange(cells, ags, tag):
                """AllGather the cells' new h slices; scatter back to hsb."""
                G = len(cells)
                gin = drp.tile([64 * G, 128], bf16, name=f"gi{tag}",
                               tag=f"gi{G}")
                gout = drp.tile([512 * G, 128], bf16, name=f"go{tag}",
                                tag=f"go{G}")
                for ci, ag in enumerate(ags):
                    nc.sync.dma_start(gin[64 * ci:64 * (ci + 1), :], ag[:, :])
                nc.gpsimd.collective_compute(
                    "AllGather", mybir.AluOpType.bypass,
                    ins=[gin.opt()], outs=[gout.opt()],
                    replica_groups=[list(range(NCORES))])
                v = gout[:, :].rearrange("(j h ci q) b -> ci h q j b",
                                         j=4, h=2, ci=G, q=64)
                for ci, cell in enumerate(cells):
                    for hh in (0, 1):
                        nc.sync.dma_start(
                            hsb[cell][64 * hh:64 * (hh + 1), :].rearrange(
                                "q (j b) -> q j b", j=4),
                            v[ci, hh])

            def stage(cells_rhs, tag):
                psums = []
                for ci, cr in enumerate(cells_rhs):
                    cell, rhs = cr[0], cr[1]
                    order = cr[2] if len(cr) > 2 else None
                    ps = psp.tile([128, 256], f32, name=f"ps{tag}{ci}",
                                  tag="ps")
                    cell_mms(ps, cell, rhs, order)
                    psums.append(ps)
                ags = []
                for ci, cr in enumerate(cells_rhs):
                    ags.append(cell_ew(psums[ci], cr[0], f"{tag}{ci}"))
                exchange([cr[0] for cr in cells_rhs], ags, tag)

            def fc(stack, t):
                htop = hsb["c2" if stack == "c" else "d2"]
                psf = psp.tile([128, 256], f32, name="fcps", tag="ps")
                for m in (0, 1):
                    for kt in range(4):
                        col = (kt * 2 + m) * 128
                        nc.tensor.matmul(
                            psf[:, 128 * m:128 * (m + 1)],
                            fcwsb[stack][:, col:col + 128],
                            htop[:, 128 * kt:128 * (kt + 1)],
                            start=(kt == 0), stop=(kt == 3))
                ot = osbp.tile([128, 256], f32, name="fcout", tag="fcout")
                for m in (0, 1):
                    nc.vector.tensor_scalar_add(
                        ot[:, 128 * m:128 * (m + 1)],
                        psf[:, 128 * m:128 * (m + 1)],
                        fcbsb[stack][:, m:m + 1])
                nc.sync.dma_start(
                    zout[stack].ap()[t].rearrange("(m p) b -> p m b", m=2),
                    ot[:, :].rearrange("p (m b) -> p m b", m=2))

            for t in range(T):
                xct = xp.tile([128, 128], bf16, name="xc", tag="xc")
                xdt = xp.tile([128, 128], bf16, name="xd", tag="xd")
                nc.sync.dma_start(xct[:, :], xc.ap()[t])
                nc.sync.dma_start(xdt[:, :], xd.ap()[t])

                # A: d1(t-1), c0(t)
                a_cells = []
                if t >= 1:
                    a_cells.append(("d1", h_tiles("d0") + h_tiles("d1")
                                    + h_tiles("c1")))
                a_cells.append(("c0",
                                [xct[:, :]] + h_tiles("c0") + h_tiles("d0")))
                stage(a_cells, "A")
                if t >= 2:
                    fc("d", t - 2)
                # B1: c2(t-1)
                if t >= 1:
                    stage([("c2", h_tiles("c1") + h_tiles("c2")
                            + h_tiles("d2"))], "B")
                # C: c1(t), d0(t)
                stage([("d0", [xdt[:, :]] + h_tiles("d0") + h_tiles("c0")),
                       ("c1", h_tiles("c0") + h_tiles("c1") + h_tiles("d1"),
                        [4, 5, 6, 7, 0, 1, 2, 3, 8, 9, 10, 11])],
                      "C")
                # B2: d2(t-1)
                if t >= 1:
                    stage([("d2", h_tiles("d1") + h_tiles("d2")
                            + h_tiles("c2"))], "D")
                    fc("c", t - 1)

            # epilogue: d1(T-1), c2(T-1), d2(T-1) and last FCs
            stage([("d1", h_tiles("d0") + h_tiles("d1") + h_tiles("c1")),
                   ("c2", h_tiles("c1") + h_tiles("c2") + h_tiles("d2"))],
                  "E1")
            if T >= 2:
                fc("d", T - 2)
            stage([("d2", h_tiles("d1") + h_tiles("d2") + h_tiles("c2"))],
                  "E2")
            fc("c", T - 1)
            fc("d", T - 1)

    nc.compile()
    return nc


# ---------------- host side ----------------

def _prep_w_chunk(W, k):
    # rows: m0 = [f|i] for dims [64k,64k+64); m1 = [o|c~]
    r = np.arange(64 * k, 64 * k + 64)
    rows = np.concatenate([512 + r, r, 1024 + r, 1536 + r])
    Wk = W[rows, :]                      # (256, K)
    K = Wk.shape[1]
    nk = K // 128
    lhsT = Wk.T.reshape(nk, 128, 2, 128)  # [kt, p, m, j]
    return np.ascontiguousarray(lhsT.astype(ml_dtypes.bfloat16))


_CACHE = {}


def _run_device(noise_c, noise_d, Ws, fc_w, fc_b, T, trace=False):
    if T not in _CACHE:
        _CACHE[T] = build_kernel(T)
    nc = _CACHE[T]

    xc_h = np.ascontiguousarray(
        noise_c.transpose(1, 2, 0).astype(ml_dtypes.bfloat16))
    xd_h = np.ascontiguousarray(
        noise_d.transpose(1, 2, 0).astype(ml_dtypes.bfloat16))

    fcw_h = {}
    fcb_h = {}
    for s in "cd":
        fcw_h[s] = np.ascontiguousarray(
            fc_w[s].T.reshape(4, 128, 2, 128).astype(ml_dtypes.bfloat16))
        fcb_h[s] = np.ascontiguousarray(
            fc_b[s].reshape(2, 128).T.astype(np.float32))

    ridm_h = np.zeros((128, 64), np.float32)
    ridm_h[np.arange(128), np.arange(128) % 64] = 1.0
    in_maps = []
    for k in range(NCORES):
        m = {"xc": xc_h, "xd": xd_h, "ridm": ridm_h}
        for c in CELLS:
            m[f"w_{c}"] = _prep_w_chunk(Ws[c], k)
        for s in "cd":
            m[f"fcw_{s}"] = fcw_h[s]
            m[f"fcb_{s}"] = fcb_h[s]
        in_maps.append(m)

    res = run_bass_kernel_spmd(nc, in_maps, core_ids=list(range(NCORES)),
                               trace=trace)
    out = {}
    for s in "cd":
        z = res.results[0][f"z_{s}"]          # (T, 256, B)
        out[s] = np.ascontiguousarray(z.transpose(2, 0, 1)).astype(np.float32)
    return out["c"], out["d"], res


def _np_reference(noise_c, noise_d, inp):
    def cell(x, hs, cs, hc, W):
        g = np.concatenate([x, hs, hc], axis=1) @ W.T
        i, f, o, ct = np.split(g, 4, axis=1)
        sig = lambda v: 1.0 / (1.0 + np.exp(-v))
        cn = sig(f) * cs + sig(i) * np.tanh(ct)
        hn = sig(o) * np.tanh(cn)
        return hn, cn

    Bn, Tn = noise_c.shape[0], noise_c.shape[1]
    ch = [np.zeros((Bn, H), np.float32) for _ in range(3)]
    cc = [np.zeros((Bn, H), np.float32) for _ in range(3)]
    dh = [np.zeros((Bn, H), np.float32) for _ in range(3)]
    dc = [np.zeros((Bn, H), np.float32) for _ in range(3)]
    c_seq = np.zeros((Bn, Tn, H), np.float32)
    d_seq = np.zeros((Bn, Tn, H), np.float32)
    for t in range(Tn):
        x = noise_c[:, t]
        nch, ncc = [], []
        for i in range(3):
            h, c = cell(x, ch[i], cc[i], dh[i], inp[f"c_W{i}"])
            nch.append(h); ncc.append(c); x = h
        c_seq[:, t] = x
        x = noise_d[:, t]
        ndh, ndc = [], []
        for i in range(3):
            h, c = cell(x, dh[i], dc[i], nch[i], inp[f"d_W{i}"])
            ndh.append(h); ndc.append(c); x = h
        d_seq[:, t] = x
        ch, cc, dh, dc = nch, ncc, ndh, ndc

    def attn(x, qw, qb, kw, kb, vw, vb, gamma):
        b, t, h = x.shape
        pq = (x @ qw.T + qb).reshape(b, -1, t).transpose(0, 2, 1)
        pk = (x @ kw.T + kb).reshape(b, -1, t)
        e = np.einsum('btk,bks->bts', pq, pk)
        e = e - e.max(-1, keepdims=True)
        a = np.exp(e); a = a / a.sum(-1, keepdims=True)
        pv = (x @ vw.T + vb).reshape(b, -1, t)
        o = np.einsum('bht,bst->bhs', pv, a).reshape(b, t, h)
        return gamma * o + x

    c_a = attn(c_seq, inp["c_q_w"], inp["c_q_b"], inp["c_k_w"], inp["c_k_b"],
               inp["c_v_w"], inp["c_v_b"], inp["c_gamma"])
    d_a = attn(d_seq, inp["d_q_w"], inp["d_q_b"], inp["d_k_w"], inp["d_k_b"],
               inp["d_v_w"], inp["d_v_b"], inp["d_gamma"])
    zc = c_a @ inp["c_fc_w"].T + inp["c_fc_b"]
    zd = d_a @ inp["d_fc_w"].T + inp["d_fc_b"]
    return zc.astype(np.float32), zd.astype(np.float32)


def kernel(**inputs):
    inp = {k: np.asarray(v) for k, v in inputs.items()}
    if np.any(inp["c_gamma"] != 0) or np.any(inp["d_gamma"] != 0):
        return _np_reference(inp["noise_c"].astype(np.float32),
                             inp["noise_d"].astype(np.float32), inp)

    Ws = {f"{s}{i}": inp[f"{s}_W{i}"].astype(np.float32)
          for s in "cd" for i in range(3)}
    fc_w = {s: inp[f"{s}_fc_w"].astype(np.float32) for s in "cd"}
    fc_b = {s: inp[f"{s}_fc_b"].astype(np.float32) for s in "cd"}
    zc, zd, _ = _run_device(inp["noise_c"].astype(np.float32),
                            inp["noise_d"].astype(np.float32),
                            Ws, fc_w, fc_b, inp["noise_c"].shape[1])
    return zc, zd
